# revision 6
# baseline (speedup 1.0000x reference)
"""Trainium2 Bass kernel for nn_AttnFathers.

Reference computation:
    energy      = einsum('bmfh,kh->bmfk', FO, W) + bias          # huge matmul
    attn_energy = einsum('bh,bmfh->bmf', hidden[0], energy)
    out         = softmax(attn_energy, axis=1)                   # over m

Algebraic rewrite (exact, in fp32):
    v[b]   = hidden[b] @ W          # [H]
    c[b]   = hidden[b] . bias       # scalar, cancels in softmax
    e[b,m,f] = FO[b,m,f,:].v[b] + c[b]
    out    = softmax_m(e)

fp16 edition: FO/hid/W cast to fp16 on the host (rel err ~7e-3, within
the 2e-2 gate) -> 33.5 MB HBM per core, ~100us DMA floor.

Engine plan (the fused multiply+row-reduce InstTensorScalarPtr has NO
DVE fast mode; TensorTensor mult does have 2x_1p for fp16; the backend
rejects TensorScalarPtr/TensorReduce on Pool but accepts TensorTensor):
  FO streams as 64 paired slices [128, 2, 1024] (512KB DMAs, halving
  dma_start sequencer cost and semaphore traffic). Per pair one of:
    A: 2x fused scalar_tensor_tensor on DVE          (~1.3us/tile)
    C: one pair-mult on DVE (2x_1p, ~1.2us/pair)
       + 2 activation-Copy reduces on ACT            (~1.1us/tile)
    E: one pair-mult on Pool (~4.1us/pair)
       + 2 activation-Copy reduces on ACT
  A:C:E = 30:10:24 pairs -> DVE/ACT/Pool all ~100us, hidden under DMA.

Sharding: data-parallel over batch B=16 -> 2 batches per core on 8 cores.
"""

import sys
import os

for _p in ("/opt/trn_rl_repo", "/root/.axon_site/_ro/trn_rl_repo"):
    if os.path.isdir(_p) and _p not in sys.path:
        sys.path.insert(0, _p)

import numpy as np
from contextlib import ExitStack

import concourse.bass as bass
import concourse.bacc as bacc
import concourse.tile as tile
from concourse import mybir
from concourse.bass_utils import run_bass_kernel_spmd

F32 = mybir.dt.float32
F16 = mybir.dt.float16

B, MAX_LEN, FATHER_NUM, H = 16, 256, 32, 1024
NCORES = 8
BPC = B // NCORES                 # batches per core = 2
ROWS = MAX_LEN * FATHER_NUM       # rows per batch = 8192 (r = m*32 + f)
P = 128
TPB = ROWS // P                   # 128-row tiles per batch = 64
UPB = TPB // 2                    # 256-row pairs per batch = 32
SLICE_BUFS = 14                   # in-flight 512KB fp16 pair slices (7 MB)
PROD_BUFS = 6
KC = H // P                       # 128-contraction chunks = 8

# Per-pair engine path assignment (counts tuned to balance engine busy):
#   'A' = 2x DVE fused, 'C' = DVE pair-mult + ACT reduces,
#   'E' = Pool pair-mult + ACT reduces
PATH_COUNTS = {"A": 30, "C": 10, "E": 24}


def _make_paths():
    n = 2 * UPB
    acc = {k: 0.0 for k in PATH_COUNTS}
    out = []
    for _ in range(n):
        for k in PATH_COUNTS:
            acc[k] += PATH_COUNTS[k] / n
        k = max(acc, key=lambda x: acc[x])
        acc[k] -= 1.0
        out.append(k)
    return out


PATHS = _make_paths()


def build_nc() -> bass.Bass:
    nc = bacc.Bacc(trn_type="TRN2")

    # NOTE: attn_b is deliberately absent. The bias contributes
    # hidden[b].attn_b to every logit of batch b — constant across the
    # softmax axis m (and f), so it cancels exactly in the softmax.
    fo = nc.dram_tensor("fo", [BPC, MAX_LEN, FATHER_NUM, H], F16, kind="ExternalInput")
    hidT = nc.dram_tensor("hidT", [H, BPC], F16, kind="ExternalInput")
    w = nc.dram_tensor("w", [H, H], F16, kind="ExternalInput")
    out = nc.dram_tensor("out", [BPC, MAX_LEN, FATHER_NUM], F32, kind="ExternalOutput")

    # Constant tensors embedded in the NEFF.
    ident_np = np.eye(P, dtype=np.float32)
    # gmat[p, f] = 1 if p % 32 == f  (sums the 4 partition groups -> 32 f-rows)
    gmat_np = np.zeros((P, FATHER_NUM), dtype=np.float32)
    gmat_np[np.arange(P), np.arange(P) % FATHER_NUM] = 1.0
    # g2[f, p] = 1 if p % 32 == f    (broadcast 32 f-rows -> 128 partitions)
    g2_np = np.ascontiguousarray(gmat_np.T)
    # bsel[k, b*128 + p] = 1 if k == b (broadcast row b of a [BPC, N] tensor)
    bsel_np = np.zeros((BPC, BPC * P), dtype=np.float16)
    for b in range(BPC):
        bsel_np[b, b * P:(b + 1) * P] = 1.0
    negones_np = np.full((1, P), -1.0, dtype=np.float32)

    ident_d = nc.inline_tensor(ident_np, "identc")
    gmat_d = nc.inline_tensor(gmat_np, "gmatc")
    g2_d = nc.inline_tensor(g2_np, "g2c")
    bsel_d = nc.inline_tensor(bsel_np, "bselc")
    negones_d = nc.inline_tensor(negones_np, "negonesc")

    with tile.TileContext(nc) as tc, ExitStack() as ctx:
        consts = ctx.enter_context(tc.tile_pool(name="consts", bufs=1))
        wpool = ctx.enter_context(tc.tile_pool(name="wpool", bufs=1))
        chunks = ctx.enter_context(tc.tile_pool(name="chunks", bufs=SLICE_BUFS))
        prods = ctx.enter_context(tc.tile_pool(name="prods", bufs=PROD_BUFS))
        scratchp = ctx.enter_context(tc.tile_pool(name="scratchp", bufs=1))
        epool = ctx.enter_context(tc.tile_pool(name="epool", bufs=2))
        smallp = ctx.enter_context(tc.tile_pool(name="smallp", bufs=2))
        outp = ctx.enter_context(tc.tile_pool(name="outp", bufs=2))
        psum1 = ctx.enter_context(tc.tile_pool(name="psum1", bufs=2, space="PSUM"))
        psum2 = ctx.enter_context(tc.tile_pool(name="psum2", bufs=1, space="PSUM"))

        # ---- urgent inputs first: hidT (tiny), then W split across both
        # HWDGE rings so the v-chain completes while FO slices queue behind.
        hT = consts.tile([P, KC, BPC], F16)
        nc.sync.dma_start(
            out=hT, in_=hidT.ap().rearrange("(k p) b -> p k b", k=KC, p=P)
        )

        # W as [k-partition, kc, h] so rhs chunks are wt[:, kc, n0:n1].
        wt = wpool.tile([P, KC, H], F16)
        w_ap = w.ap()
        for k in range(KC):
            eng = nc.sync if k % 2 == 0 else nc.scalar
            eng.dma_start(out=wt[:, k, :], in_=w_ap[k * P:(k + 1) * P, :])

        # Softmax constants (not urgent).
        ident = consts.tile([P, P], F32)
        nc.scalar.dma_start(out=ident, in_=ident_d.ap())
        gmat = consts.tile([P, FATHER_NUM], F32)
        nc.scalar.dma_start(out=gmat, in_=gmat_d.ap())
        g2 = consts.tile([FATHER_NUM, P], F32)
        nc.sync.dma_start(out=g2, in_=g2_d.ap())
        bsel = consts.tile([BPC, BPC * P], F16)
        nc.sync.dma_start(out=bsel, in_=bsel_d.ap())
        negones = consts.tile([1, P], F32)
        nc.scalar.dma_start(out=negones, in_=negones_d.ap())

        # Warm the ACT exp table and the Pool mult path during the prologue.
        warm = consts.tile([1, 1], F32)
        nc.vector.memset(warm, 0.0)
        nc.scalar.activation(
            out=warm, in_=warm, func=mybir.ActivationFunctionType.Exp
        )
        warm2 = consts.tile([1, 2], F16)
        nc.gpsimd.memset(warm2, 0.0)
        nc.gpsimd.tensor_tensor(
            out=warm2, in0=warm2, in1=warm2, op=mybir.AluOpType.mult
        )

        # ---- v = hidden @ W  -> [BPC, H] -----------------------------------
        v_ps = psum2.tile([BPC, H], F32, tag="ps2")
        for k in range(KC):  # k outer: consume each W chunk as it lands
            for half in range(2):
                n0, n1 = half * 512, (half + 1) * 512
                nc.tensor.matmul(
                    v_ps[:, n0:n1], hT[:, k, :], wt[:, k, n0:n1],
                    start=(k == 0), stop=(k == KC - 1),
                )
        v_sb = consts.tile([BPC, H], F16)
        nc.vector.tensor_copy(out=v_sb, in_=v_ps)

        # ---- per-batch broadcast: v[b] -> [128, H] fp16 --------------------
        vbc = []
        for b in range(BPC):
            vb_ps = psum2.tile([P, H], F32, tag="ps2")
            for half in range(2):
                n0, n1 = half * 512, (half + 1) * 512
                nc.tensor.matmul(
                    vb_ps[:, n0:n1], bsel[:, b * P:(b + 1) * P], v_sb[:, n0:n1],
                    start=True, stop=True,
                )
            vbc_b = consts.tile([P, H], F16, tag=f"vbc{b}")
            nc.vector.tensor_copy(out=vbc_b, in_=vb_ps)
            vbc.append(vbc_b)
        # vbc2[b] = [128, 2, 1024] view of v broadcast for pair-mults
        vbc2 = []
        for b in range(BPC):
            vb2 = consts.tile([P, 2, H], F16, tag=f"vbc2{b}")
            nc.vector.tensor_copy(out=vb2[:, 0, :], in_=vbc[b])
            nc.vector.tensor_copy(out=vb2[:, 1, :], in_=vbc[b])
            vbc2.append(vb2)

        # ---- main loop: stream FO per 512KB fp16 pair slice ----------------
        # row r = m*32 + f of FO[b]; pair u covers rows [u*256, (u+1)*256):
        # tile q=0 -> rows u*256+p, q=1 -> rows u*256+128+p
        fo_r = (
            fo.ap()
            .flatten_outer_dims()  # [BPC*ROWS, H]
            .rearrange("(b u q p) h -> b u p q h", b=BPC, u=UPB, q=2, p=P)
        )
        out_r = (
            out.ap()
            .rearrange("b m f -> b (m f)")
            .rearrange("b (t p) -> b t p", t=TPB, p=P)
        )

        scr_dve = scratchp.tile([P, H], F16, tag="scr_dve")
        scr_act = scratchp.tile([P, H], F16, tag="scr_act")

        def emit_pair(b, u, e_t):
            i = b * UPB + u
            ck2 = chunks.tile([P, 2, H], F16, tag="ck")
            eng = nc.sync if i % 2 == 0 else nc.scalar
            eng.dma_start(out=ck2, in_=fo_r[b, u])
            t0 = 2 * u
            path = PATHS[i]
            if path == "A":
                for q in range(2):
                    nc.vector.scalar_tensor_tensor(
                        out=scr_dve, in0=ck2[:, q, :], scalar=1.0, in1=vbc[b],
                        op0=mybir.AluOpType.bypass, op1=mybir.AluOpType.mult,
                        accum_out=e_t[:, t0 + q:t0 + q + 1],
                    )
            else:
                pr2 = prods.tile([P, 2, H], F16, tag="pr")
                eng2 = nc.vector if path == "C" else nc.gpsimd
                eng2.tensor_tensor(
                    out=pr2, in0=ck2, in1=vbc2[b], op=mybir.AluOpType.mult
                )
                for q in range(2):
                    nc.scalar.activation(
                        out=scr_act, in_=pr2[:, q, :],
                        func=mybir.ActivationFunctionType.Copy,
                        accum_out=e_t[:, t0 + q:t0 + q + 1],
                    )

        def softmax_emit(b, e_t):
            # Softmax over m (free axis t + partition groups of 32).
            # Batch-global max K: constant shift per batch, valid for
            # softmax over m at every f.
            colmax = smallp.tile([P, 1], F32, tag="cmax")
            nc.vector.reduce_max(out=colmax, in_=e_t, axis=mybir.AxisListType.X)
            cm_ps = psum1.tile([1, P], F32, tag="ps1")
            nc.tensor.transpose(cm_ps, colmax, ident)
            gmax = smallp.tile([1, 1], F32, tag="gmax")
            nc.vector.reduce_max(out=gmax, in_=cm_ps, axis=mybir.AxisListType.X)

            negK_ps = psum1.tile([P, 1], F32, tag="ps1")
            nc.tensor.matmul(negK_ps, negones, gmax, start=True, stop=True)
            negK = smallp.tile([P, 1], F32, tag="negK")
            nc.scalar.copy(out=negK, in_=negK_ps)

            p_t = smallp.tile([P, TPB], F32, tag="pt")
            s_col = smallp.tile([P, 1], F32, tag="scol")
            nc.scalar.activation(
                out=p_t, in_=e_t,
                func=mybir.ActivationFunctionType.Exp,
                bias=negK, scale=1.0,
                accum_out=s_col,
            )

            s4_ps = psum1.tile([FATHER_NUM, 1], F32, tag="ps1")
            nc.tensor.matmul(s4_ps, gmat, s_col, start=True, stop=True)
            rinv = smallp.tile([FATHER_NUM, 1], F32, tag="rinv")
            nc.vector.reciprocal(out=rinv, in_=s4_ps)

            rb_ps = psum1.tile([P, 1], F32, tag="ps1")
            nc.tensor.matmul(rb_ps, g2, rinv, start=True, stop=True)

            nc.vector.tensor_scalar_mul(out=p_t, in0=p_t, scalar1=rb_ps)

            pT_ps = psum1.tile([TPB, P], F32, tag="pT")
            nc.tensor.transpose(pT_ps, p_t, ident)
            pT_sb = outp.tile([TPB, P], F32, tag="pTs")
            nc.scalar.copy(out=pT_sb, in_=pT_ps)
            nc.scalar.dma_start(out=out_r[b], in_=pT_sb)

        def softmax_segments(b, e_t):
            st = {}

            def seg1():  # colmax (V) + transpose (PE)
                colmax = smallp.tile([P, 1], F32, tag="cmax")
                st['colmax'] = colmax
                nc.vector.reduce_max(out=colmax, in_=e_t,
                                     axis=mybir.AxisListType.X)
                cm_ps = psum1.tile([1, P], F32, tag="ps1")
                st['cm_ps'] = cm_ps
                nc.tensor.transpose(cm_ps, colmax, ident)

            def seg2():  # global max (V), -K broadcast (PE+A), exp (A), s4 (PE)
                gmax = smallp.tile([1, 1], F32, tag="gmax")
                st['gmax'] = gmax
                nc.vector.reduce_max(out=gmax, in_=st['cm_ps'],
                                     axis=mybir.AxisListType.X)
                negK_ps = psum1.tile([P, 1], F32, tag="ps1")
                nc.tensor.matmul(negK_ps, negones, gmax, start=True, stop=True)
                negK = smallp.tile([P, 1], F32, tag="negK")
                nc.scalar.copy(out=negK, in_=negK_ps)
                p_t = smallp.tile([P, TPB], F32, tag="pt")
                s_col = smallp.tile([P, 1], F32, tag="scol")
                st['p_t'] = p_t
                nc.scalar.activation(
                    out=p_t, in_=e_t,
                    func=mybir.ActivationFunctionType.Exp,
                    bias=negK, scale=1.0,
                    accum_out=s_col,
                )
                s4_ps = psum1.tile([FATHER_NUM, 1], F32, tag="ps1")
                st['s4_ps'] = s4_ps
                nc.tensor.matmul(s4_ps, gmat, s_col, start=True, stop=True)

            def seg3():  # reciprocal (V) + broadcast (PE)
                rinv = smallp.tile([FATHER_NUM, 1], F32, tag="rinv")
                nc.vector.reciprocal(out=rinv, in_=st['s4_ps'])
                rb_ps = psum1.tile([P, 1], F32, tag="ps1")
                st['rb_ps'] = rb_ps
                nc.tensor.matmul(rb_ps, g2, rinv, start=True, stop=True)

            def seg4():  # normalize (V), transpose (PE), copy (A), store
                nc.vector.tensor_scalar_mul(out=st['p_t'], in0=st['p_t'],
                                            scalar1=st['rb_ps'])
                pT_ps = psum1.tile([TPB, P], F32, tag="pT")
                nc.tensor.transpose(pT_ps, st['p_t'], ident)
                pT_sb = outp.tile([TPB, P], F32, tag="pTs")
                nc.scalar.copy(out=pT_sb, in_=pT_ps)
                nc.scalar.dma_start(out=out_r[b], in_=pT_sb)

            return [seg1, seg2, seg3, seg4]

        # Batch 0 pairs; its softmax segments hide inside batch 1's stream.
        e_t0 = epool.tile([P, TPB], F32, tag="e")
        for u in range(UPB):
            emit_pair(0, u, e_t0)
        segs0 = softmax_segments(0, e_t0)
        seg_at = {2: 0, 5: 1, 8: 2, 11: 3}
        e_t1 = epool.tile([P, TPB], F32, tag="e")
        for u in range(UPB):
            emit_pair(1, u, e_t1)
            if u in seg_at:
                segs0[seg_at[u]]()
        # Batch 1 softmax: compact tail.
        softmax_emit(1, e_t1)

    nc.compile()
    return nc


_NC_CACHE = None


def _get_nc():
    global _NC_CACHE
    if _NC_CACHE is None:
        _NC_CACHE = build_nc()
    return _NC_CACHE


def _make_in_maps(hidden, fathers_outputs, attn_W, attn_b):
    hidden = np.asarray(hidden, dtype=np.float32)
    fo16 = np.asarray(fathers_outputs, dtype=np.float32).astype(np.float16)
    w16 = np.ascontiguousarray(np.asarray(attn_W, dtype=np.float32).astype(np.float16))
    in_maps = []
    for i in range(NCORES):
        b0 = i * BPC
        in_maps.append({
            "fo": np.ascontiguousarray(fo16[b0:b0 + BPC]),
            "hidT": np.ascontiguousarray(
                hidden[0, b0:b0 + BPC].T.astype(np.float16)
            ),
            "w": w16,
        })
    return in_maps


def run(hidden, fathers_outputs, fathers_lengths, attn_W, attn_b, trace=False):
    """Run on the 8 NeuronCores; returns (full_output, BassKernelResults)."""
    nc = _get_nc()
    in_maps = _make_in_maps(hidden, fathers_outputs, attn_W, attn_b)
    res = run_bass_kernel_spmd(nc, in_maps, list(range(NCORES)), trace=trace)
    parts = [np.asarray(res.results[i]["out"]) for i in range(NCORES)]
    full = np.concatenate(parts, axis=0).astype(np.float32)
    return full, res


def kernel(hidden, fathers_outputs, fathers_lengths, attn_W, attn_b):
    full, _ = run(hidden, fathers_outputs, fathers_lengths, attn_W, attn_b)
    return full


# revision 8
# speedup vs baseline: 1.7823x; 1.7823x over previous
"""Trainium2 Bass kernel for nn_AttnFathers.

Reference computation:
    energy      = einsum('bmfh,kh->bmfk', FO, W) + bias          # huge matmul
    attn_energy = einsum('bh,bmfh->bmf', hidden[0], energy)
    out         = softmax(attn_energy, axis=1)                   # over m

Algebraic rewrite (exact, in fp32):
    v[b]   = hidden[b] @ W          # [H]
    c[b]   = hidden[b] . bias       # scalar, cancels in softmax
    e[b,m,f] = FO[b,m,f,:].v[b] + c[b]
    out    = softmax_m(e)

fp16 edition: FO/hid/W cast to fp16 on the host (rel err ~7e-3, within
the 2e-2 gate) -> 33.5 MB HBM per core, ~100us DMA floor.

Engine plan (the fused multiply+row-reduce InstTensorScalarPtr has NO
DVE fast mode; TensorTensor mult does have 2x_1p for fp16; the backend
rejects TensorScalarPtr/TensorReduce on Pool but accepts TensorTensor):
  FO streams as 64 paired slices [128, 2, 1024] (512KB DMAs, halving
  dma_start sequencer cost and semaphore traffic). Per pair one of:
    A: 2x fused scalar_tensor_tensor on DVE          (~1.3us/tile)
    C: one pair-mult on DVE (2x_1p, ~1.2us/pair)
       + 2 activation-Copy reduces on ACT            (~1.1us/tile)
    E: one pair-mult on Pool (~4.1us/pair)
       + 2 activation-Copy reduces on ACT
  A:C:E = 30:10:24 pairs -> DVE/ACT/Pool all ~100us, hidden under DMA.

Sharding: data-parallel over batch B=16 -> 2 batches per core on 8 cores.
"""

import sys
import os

for _p in ("/opt/trn_rl_repo", "/root/.axon_site/_ro/trn_rl_repo"):
    if os.path.isdir(_p) and _p not in sys.path:
        sys.path.insert(0, _p)

import numpy as np
from contextlib import ExitStack

import concourse.bass as bass
import concourse.bacc as bacc
import concourse.tile as tile
from concourse import mybir
from concourse.bass_utils import run_bass_kernel_spmd

F32 = mybir.dt.float32
F16 = mybir.dt.float16

B, MAX_LEN, FATHER_NUM, H = 16, 256, 32, 1024
NCORES = 8
BPC = B // NCORES                 # batches per core = 2
ROWS = MAX_LEN * FATHER_NUM       # rows per batch = 8192 (r = m*32 + f)
P = 128
TPB = ROWS // P                   # 128-row tiles per batch = 64
UPB = TPB // 2                    # 256-row pairs per batch = 32
SLICE_BUFS = 14                   # in-flight 512KB fp16 pair slices (7 MB)
PROD_BUFS = 6
KC = H // P                       # 128-contraction chunks = 8

# Per-pair engine path assignment (counts tuned to balance engine busy).
# Pool is deliberately absent from the main loop: gpsimd shares SBUF R/W
# ports with DVE, and a concurrent Pool mult stream slowed DVE ops ~60%
# (measured), negating its contribution.
#   'A' = 2x DVE fused, 'C' = DVE pair-mult + ACT reduces
PATH_COUNTS = {"A": 29, "C": 35}


def _make_paths():
    n = 2 * UPB
    acc = {k: 0.0 for k in PATH_COUNTS}
    out = []
    for _ in range(n):
        for k in PATH_COUNTS:
            acc[k] += PATH_COUNTS[k] / n
        k = max(acc, key=lambda x: acc[x])
        acc[k] -= 1.0
        out.append(k)
    return out


PATHS = _make_paths()


def build_nc() -> bass.Bass:
    nc = bacc.Bacc(trn_type="TRN2")

    # NOTE: attn_b is deliberately absent. The bias contributes
    # hidden[b].attn_b to every logit of batch b — constant across the
    # softmax axis m (and f), so it cancels exactly in the softmax.
    fo = nc.dram_tensor("fo", [BPC, MAX_LEN, FATHER_NUM, H], F16, kind="ExternalInput")
    hidT = nc.dram_tensor("hidT", [H, BPC], F16, kind="ExternalInput")
    w = nc.dram_tensor("w", [H, H], F16, kind="ExternalInput")
    out = nc.dram_tensor("out", [BPC, MAX_LEN, FATHER_NUM], F32, kind="ExternalOutput")

    # Constant tensors embedded in the NEFF.
    ident_np = np.eye(P, dtype=np.float32)
    # gmat[p, f] = 1 if p % 32 == f  (sums the 4 partition groups -> 32 f-rows)
    gmat_np = np.zeros((P, FATHER_NUM), dtype=np.float32)
    gmat_np[np.arange(P), np.arange(P) % FATHER_NUM] = 1.0
    # g2[f, p] = 1 if p % 32 == f    (broadcast 32 f-rows -> 128 partitions)
    g2_np = np.ascontiguousarray(gmat_np.T)
    # bsel[k, b*128 + p] = 1 if k == b (broadcast row b of a [BPC, N] tensor)
    bsel_np = np.zeros((BPC, BPC * P), dtype=np.float16)
    for b in range(BPC):
        bsel_np[b, b * P:(b + 1) * P] = 1.0
    negones_np = np.full((1, P), -1.0, dtype=np.float32)

    ident_d = nc.inline_tensor(ident_np, "identc")
    gmat_d = nc.inline_tensor(gmat_np, "gmatc")
    g2_d = nc.inline_tensor(g2_np, "g2c")
    bsel_d = nc.inline_tensor(bsel_np, "bselc")
    negones_d = nc.inline_tensor(negones_np, "negonesc")

    with tile.TileContext(nc) as tc, ExitStack() as ctx:
        consts = ctx.enter_context(tc.tile_pool(name="consts", bufs=1))
        wpool = ctx.enter_context(tc.tile_pool(name="wpool", bufs=1))
        chunks = ctx.enter_context(tc.tile_pool(name="chunks", bufs=SLICE_BUFS))
        prods = ctx.enter_context(tc.tile_pool(name="prods", bufs=PROD_BUFS))
        scratchp = ctx.enter_context(tc.tile_pool(name="scratchp", bufs=1))
        epool = ctx.enter_context(tc.tile_pool(name="epool", bufs=2))
        smallp = ctx.enter_context(tc.tile_pool(name="smallp", bufs=2))
        outp = ctx.enter_context(tc.tile_pool(name="outp", bufs=2))
        psum1 = ctx.enter_context(tc.tile_pool(name="psum1", bufs=2, space="PSUM"))
        psum2 = ctx.enter_context(tc.tile_pool(name="psum2", bufs=1, space="PSUM"))

        # ---- urgent inputs first: hidT (tiny), then W split across both
        # HWDGE rings so the v-chain completes while FO slices queue behind.
        hT = consts.tile([P, KC, BPC], F16)
        nc.sync.dma_start(
            out=hT, in_=hidT.ap().rearrange("(k p) b -> p k b", k=KC, p=P)
        )

        # W as [k-partition, kc, h] so rhs chunks are wt[:, kc, n0:n1].
        wt = wpool.tile([P, KC, H], F16)
        w_ap = w.ap()
        for k in range(KC):
            eng = nc.sync if k % 2 == 0 else nc.scalar
            eng.dma_start(out=wt[:, k, :], in_=w_ap[k * P:(k + 1) * P, :])

        # Softmax constants (not urgent).
        ident = consts.tile([P, P], F32)
        nc.scalar.dma_start(out=ident, in_=ident_d.ap())
        gmat = consts.tile([P, FATHER_NUM], F32)
        nc.scalar.dma_start(out=gmat, in_=gmat_d.ap())
        g2 = consts.tile([FATHER_NUM, P], F32)
        nc.sync.dma_start(out=g2, in_=g2_d.ap())
        bsel = consts.tile([BPC, BPC * P], F16)
        nc.sync.dma_start(out=bsel, in_=bsel_d.ap())
        negones = consts.tile([1, P], F32)
        nc.scalar.dma_start(out=negones, in_=negones_d.ap())

        # Warm the ACT exp table and the Pool mult path during the prologue.
        warm = consts.tile([1, 1], F32)
        nc.vector.memset(warm, 0.0)
        nc.scalar.activation(
            out=warm, in_=warm, func=mybir.ActivationFunctionType.Exp
        )
        warm2 = consts.tile([1, 2], F16)
        nc.gpsimd.memset(warm2, 0.0)
        nc.gpsimd.tensor_tensor(
            out=warm2, in0=warm2, in1=warm2, op=mybir.AluOpType.mult
        )

        # ---- v = hidden @ W  -> [BPC, H] -----------------------------------
        v_ps = psum2.tile([BPC, H], F32, tag="ps2")
        for k in range(KC):  # k outer: consume each W chunk as it lands
            for half in range(2):
                n0, n1 = half * 512, (half + 1) * 512
                nc.tensor.matmul(
                    v_ps[:, n0:n1], hT[:, k, :], wt[:, k, n0:n1],
                    start=(k == 0), stop=(k == KC - 1),
                )
        v_sb = consts.tile([BPC, H], F16)
        nc.vector.tensor_copy(out=v_sb, in_=v_ps)

        # ---- per-batch broadcast: v[b] -> [128, H] fp16 --------------------
        vbc = []
        for b in range(BPC):
            vb_ps = psum2.tile([P, H], F32, tag="ps2")
            for half in range(2):
                n0, n1 = half * 512, (half + 1) * 512
                nc.tensor.matmul(
                    vb_ps[:, n0:n1], bsel[:, b * P:(b + 1) * P], v_sb[:, n0:n1],
                    start=True, stop=True,
                )
            vbc_b = consts.tile([P, H], F16, tag=f"vbc{b}")
            nc.vector.tensor_copy(out=vbc_b, in_=vb_ps)
            vbc.append(vbc_b)
        # vbc2[b] = [128, 2, 1024] view of v broadcast for pair-mults
        vbc2 = []
        for b in range(BPC):
            vb2 = consts.tile([P, 2, H], F16, tag=f"vbc2{b}")
            nc.vector.tensor_copy(out=vb2[:, 0, :], in_=vbc[b])
            nc.vector.tensor_copy(out=vb2[:, 1, :], in_=vbc[b])
            vbc2.append(vb2)

        # ---- main loop: stream FO per 512KB fp16 pair slice ----------------
        # row r = m*32 + f of FO[b]; pair u covers rows [u*256, (u+1)*256):
        # tile q=0 -> rows u*256+p, q=1 -> rows u*256+128+p
        fo_r = (
            fo.ap()
            .flatten_outer_dims()  # [BPC*ROWS, H]
            .rearrange("(b u q p) h -> b u p q h", b=BPC, u=UPB, q=2, p=P)
        )
        out_r = (
            out.ap()
            .rearrange("b m f -> b (m f)")
            .rearrange("b (t p) -> b t p", t=TPB, p=P)
        )

        scr_dve = scratchp.tile([P, H], F16, tag="scr_dve")
        scr_act = scratchp.tile([P, H], F16, tag="scr_act")

        def emit_pair(b, u, e_t):
            i = b * UPB + u
            ck2 = chunks.tile([P, 2, H], F16, tag="ck")
            # 5/8 of FO pair-DMAs issue on the SP ring, 3/8 on ACT's, so
            # descriptor generation (~650ns each) mostly stays off the
            # busy ACT sequencer while neither HWDGE queue saturates.
            eng = nc.sync if i % 8 < 5 else nc.scalar
            eng.dma_start(out=ck2, in_=fo_r[b, u])
            t0 = 2 * u
            path = PATHS[i]
            if path == "A":
                for q in range(2):
                    nc.vector.scalar_tensor_tensor(
                        out=scr_dve, in0=ck2[:, q, :], scalar=1.0, in1=vbc[b],
                        op0=mybir.AluOpType.bypass, op1=mybir.AluOpType.mult,
                        accum_out=e_t[:, t0 + q:t0 + q + 1],
                    )
            else:
                pr2 = prods.tile([P, 2, H], F16, tag="pr")
                nc.vector.tensor_tensor(
                    out=pr2, in0=ck2, in1=vbc2[b], op=mybir.AluOpType.mult
                )
                for q in range(2):
                    nc.scalar.activation(
                        out=scr_act, in_=pr2[:, q, :],
                        func=mybir.ActivationFunctionType.Copy,
                        accum_out=e_t[:, t0 + q:t0 + q + 1],
                    )

        def softmax_emit(b, e_t):
            # Softmax over m (free axis t + partition groups of 32).
            # Batch-global max K: constant shift per batch, valid for
            # softmax over m at every f.
            colmax = smallp.tile([P, 1], F32, tag="cmax")
            nc.vector.reduce_max(out=colmax, in_=e_t, axis=mybir.AxisListType.X)
            cm_ps = psum1.tile([1, P], F32, tag="ps1")
            nc.tensor.transpose(cm_ps, colmax, ident)
            gmax = smallp.tile([1, 1], F32, tag="gmax")
            nc.vector.reduce_max(out=gmax, in_=cm_ps, axis=mybir.AxisListType.X)

            negK_ps = psum1.tile([P, 1], F32, tag="ps1")
            nc.tensor.matmul(negK_ps, negones, gmax, start=True, stop=True)
            negK = smallp.tile([P, 1], F32, tag="negK")
            nc.scalar.copy(out=negK, in_=negK_ps)

            p_t = smallp.tile([P, TPB], F32, tag="pt")
            s_col = smallp.tile([P, 1], F32, tag="scol")
            nc.scalar.activation(
                out=p_t, in_=e_t,
                func=mybir.ActivationFunctionType.Exp,
                bias=negK, scale=1.0,
                accum_out=s_col,
            )

            s4_ps = psum1.tile([FATHER_NUM, 1], F32, tag="ps1")
            nc.tensor.matmul(s4_ps, gmat, s_col, start=True, stop=True)
            rinv = smallp.tile([FATHER_NUM, 1], F32, tag="rinv")
            nc.vector.reciprocal(out=rinv, in_=s4_ps)

            rb_ps = psum1.tile([P, 1], F32, tag="ps1")
            nc.tensor.matmul(rb_ps, g2, rinv, start=True, stop=True)

            nc.vector.tensor_scalar_mul(out=p_t, in0=p_t, scalar1=rb_ps)

            pT_ps = psum1.tile([TPB, P], F32, tag="pT")
            nc.tensor.transpose(pT_ps, p_t, ident)
            pT_sb = outp.tile([TPB, P], F32, tag="pTs")
            nc.scalar.copy(out=pT_sb, in_=pT_ps)
            nc.scalar.dma_start(out=out_r[b], in_=pT_sb)

        def softmax_segments(b, e_t):
            st = {}

            def seg1():  # colmax (V) + transpose (PE)
                colmax = smallp.tile([P, 1], F32, tag="cmax")
                st['colmax'] = colmax
                nc.vector.reduce_max(out=colmax, in_=e_t,
                                     axis=mybir.AxisListType.X)
                cm_ps = psum1.tile([1, P], F32, tag="ps1")
                st['cm_ps'] = cm_ps
                nc.tensor.transpose(cm_ps, colmax, ident)

            def seg2():  # global max (V), -K broadcast (PE+A), exp (A), s4 (PE)
                gmax = smallp.tile([1, 1], F32, tag="gmax")
                st['gmax'] = gmax
                nc.vector.reduce_max(out=gmax, in_=st['cm_ps'],
                                     axis=mybir.AxisListType.X)
                negK_ps = psum1.tile([P, 1], F32, tag="ps1")
                nc.tensor.matmul(negK_ps, negones, gmax, start=True, stop=True)
                negK = smallp.tile([P, 1], F32, tag="negK")
                nc.scalar.copy(out=negK, in_=negK_ps)
                p_t = smallp.tile([P, TPB], F32, tag="pt")
                s_col = smallp.tile([P, 1], F32, tag="scol")
                st['p_t'] = p_t
                nc.scalar.activation(
                    out=p_t, in_=e_t,
                    func=mybir.ActivationFunctionType.Exp,
                    bias=negK, scale=1.0,
                    accum_out=s_col,
                )
                s4_ps = psum1.tile([FATHER_NUM, 1], F32, tag="ps1")
                st['s4_ps'] = s4_ps
                nc.tensor.matmul(s4_ps, gmat, s_col, start=True, stop=True)

            def seg3():  # reciprocal (V) + broadcast (PE)
                rinv = smallp.tile([FATHER_NUM, 1], F32, tag="rinv")
                nc.vector.reciprocal(out=rinv, in_=st['s4_ps'])
                rb_ps = psum1.tile([P, 1], F32, tag="ps1")
                st['rb_ps'] = rb_ps
                nc.tensor.matmul(rb_ps, g2, rinv, start=True, stop=True)

            def seg4():  # normalize (V), transpose (PE), copy (A), store
                nc.vector.tensor_scalar_mul(out=st['p_t'], in0=st['p_t'],
                                            scalar1=st['rb_ps'])
                pT_ps = psum1.tile([TPB, P], F32, tag="pT")
                nc.tensor.transpose(pT_ps, st['p_t'], ident)
                pT_sb = outp.tile([TPB, P], F32, tag="pTs")
                nc.scalar.copy(out=pT_sb, in_=pT_ps)
                nc.scalar.dma_start(out=out_r[b], in_=pT_sb)

            return [seg1, seg2, seg3, seg4]

        # Batch 0 pairs; its softmax segments hide inside batch 1's stream.
        e_t0 = epool.tile([P, TPB], F32, tag="e")
        for u in range(UPB):
            emit_pair(0, u, e_t0)
        segs0 = softmax_segments(0, e_t0)
        seg_at = {2: 0, 5: 1, 8: 2, 11: 3}
        e_t1 = epool.tile([P, TPB], F32, tag="e")
        for u in range(UPB):
            emit_pair(1, u, e_t1)
            if u in seg_at:
                segs0[seg_at[u]]()
        # Batch 1 softmax: compact tail.
        softmax_emit(1, e_t1)

    nc.compile()
    return nc


_NC_CACHE = None


def _get_nc():
    global _NC_CACHE
    if _NC_CACHE is None:
        _NC_CACHE = build_nc()
    return _NC_CACHE


def _make_in_maps(hidden, fathers_outputs, attn_W, attn_b):
    hidden = np.asarray(hidden, dtype=np.float32)
    fo16 = np.asarray(fathers_outputs, dtype=np.float32).astype(np.float16)
    w16 = np.ascontiguousarray(np.asarray(attn_W, dtype=np.float32).astype(np.float16))
    in_maps = []
    for i in range(NCORES):
        b0 = i * BPC
        in_maps.append({
            "fo": np.ascontiguousarray(fo16[b0:b0 + BPC]),
            "hidT": np.ascontiguousarray(
                hidden[0, b0:b0 + BPC].T.astype(np.float16)
            ),
            "w": w16,
        })
    return in_maps


def run(hidden, fathers_outputs, fathers_lengths, attn_W, attn_b, trace=False):
    """Run on the 8 NeuronCores; returns (full_output, BassKernelResults)."""
    nc = _get_nc()
    in_maps = _make_in_maps(hidden, fathers_outputs, attn_W, attn_b)
    res = run_bass_kernel_spmd(nc, in_maps, list(range(NCORES)), trace=trace)
    parts = [np.asarray(res.results[i]["out"]) for i in range(NCORES)]
    full = np.concatenate(parts, axis=0).astype(np.float32)
    return full, res


def kernel(hidden, fathers_outputs, fathers_lengths, attn_W, attn_b):
    full, _ = run(hidden, fathers_outputs, fathers_lengths, attn_W, attn_b)
    return full
